# revision 1
# baseline (speedup 1.0000x reference)
"""DeepseekV2 MLA prefill kernel for 8 Trainium2 NeuronCores.

Sharding strategy (from the TP hint, adapted):
  Launch A: sequence-parallel fused qkv_a projection + RMSNorms.
            Each core handles T/8 = 256 tokens, computing the transposed
            (feature-major) normed q_a / kv_a and raw k_pe for its slice.
  Host:     gather token slices -> full feature-major activations.
  Launch B: tensor-parallel over heads (2 heads/core): q_b / kv_b
            projections (ColumnParallel), rope, causal attention,
            o_proj (RowParallel) producing partial outputs.
  Host:     sum the 8 partial outputs (the RowParallel all-reduce).

Matmuls run in float32r (fp32 data on the fast single-pass PE path;
1 cycle/row when the moving free dim >= 256). Operands feeding a
float32r matmul must themselves be float32r-typed (walrus rounding
rule), so matmul-feeding tiles/weights are allocated as float32r.
"""

import numpy as np
import concourse.bacc as bacc
import concourse.tile as tile
from concourse import mybir
from concourse import bass_utils

F32 = mybir.dt.float32
F32R = mybir.dt.float32r
F16 = mybir.dt.float16
BF16 = F16
AF = mybir.ActivationFunctionType
AX = mybir.AxisListType

NCORES = 8
T, HID, H = 2048, 5120, 16
NOPE, ROPE, VDIM = 128, 64, 128
QLORA, KVLORA = 1536, 512
FUSED = QLORA + KVLORA + ROPE  # 2112
TS = T // NCORES               # 256 tokens/core in launch A
HPC = H // NCORES              # 2 heads/core in launch B
EPS = 1e-6
THETA = 10000.0
SCALING = float((NOPE + ROPE) ** -0.5)
NEG = -1.0e30
KT = HID // 128                # 40
MT = (FUSED + 127) // 128      # 17 (last tile 64 rows)
TT = T // 128                  # 16
NCH = T // 512                 # 4


def _build_a():
    nc = bacc.Bacc("TRN2", target_bir_lowering=False, debug=False,
                   num_devices=NCORES)
    hid_s = nc.dram_tensor("hid_s", [TS, HID], F16,
                           kind="ExternalInput").ap()
    w_fused = nc.dram_tensor("w_fused", [HID, FUSED], F16,
                             kind="ExternalInput").ap()
    q_ln = nc.dram_tensor("q_ln", [QLORA], F32, kind="ExternalInput").ap()
    kv_ln = nc.dram_tensor("kv_ln", [KVLORA], F32, kind="ExternalInput").ap()
    ident = nc.dram_tensor("ident", [128, 128], F32, kind="ExternalInput").ap()
    q_aT_s = nc.dram_tensor("q_aT_s", [QLORA, TS], F16,
                            kind="ExternalOutput").ap()
    kv_aT_s = nc.dram_tensor("kv_aT_s", [KVLORA, TS], F16,
                             kind="ExternalOutput").ap()
    k_peT_s = nc.dram_tensor("k_peT_s", [ROPE, TS], F16,
                             kind="ExternalOutput").ap()

    with tile.TileContext(nc) as tc:
        with tc.tile_pool(name="consts", bufs=1) as consts, \
             tc.tile_pool(name="hidT_pool", bufs=1) as hidT_pool, \
             tc.tile_pool(name="qkv_pool", bufs=1) as qkv_pool, \
             tc.tile_pool(name="small", bufs=1) as small:
            ident_sb = consts.tile([128, 128], F32)
            nc.sync.dma_start(out=ident_sb, in_=ident)
            ident16 = consts.tile([128, 128], F16)
            nc.vector.tensor_copy(ident16, ident_sb)
            ones_f32 = consts.tile([128, 1], F32)
            nc.vector.memset(ones_f32, 1.0)
            ones_col = consts.tile([128, 1], F32R)
            nc.vector.tensor_copy(ones_col, ones_f32)
            ones_row_f32 = consts.tile([1, 128], F32)
            nc.vector.memset(ones_row_f32, 1.0)
            ones_row = consts.tile([1, 128], F32R)
            nc.vector.tensor_copy(ones_row, ones_row_f32)
            ln_sb = consts.tile([128, 16], F32)
            nc.sync.dma_start(out=ln_sb[:, 0:12],
                              in_=q_ln.rearrange("(a p) -> p a", p=128))
            nc.sync.dma_start(out=ln_sb[:, 12:16],
                              in_=kv_ln.rearrange("(a p) -> p a", p=128))
            eps_sb = small.tile([1, 1], F32)
            nc.vector.memset(eps_sb, EPS)

            # hidT[p, k, t] = hidden_slice[t, k*128+p]
            hidT = hidT_pool.tile([128, KT, TS], F16)
            with tc.tile_pool(name="hload", bufs=2) as hp, \
                 tc.tile_pool(name="tpsum", bufs=4, space="PSUM") as tp:
                for tt in range(TS // 128):
                    ht = hp.tile([128, HID], F16, tag="ht")
                    nc.sync.dma_start(out=ht,
                                      in_=hid_s[tt * 128:(tt + 1) * 128, :])
                    for k in range(KT):
                        ps = tp.tile([128, 128], F16, tag="tps")
                        nc.tensor.transpose(ps, ht[:, k * 128:(k + 1) * 128],
                                            ident16)
                        nc.vector.tensor_copy(
                            hidT[:, k, tt * 128:(tt + 1) * 128], ps)

            qkvT = qkv_pool.tile([128, MT, TS], F32)
            qkv16 = qkv_pool.tile([128, MT, TS], F16)
            with tc.tile_pool(name="wpool", bufs=4) as wp, \
                 tc.tile_pool(name="mpsum", bufs=3, space="PSUM") as mp, \
                 tc.tile_pool(name="sqpool", bufs=3) as sqp, \
                 tc.tile_pool(name="sumps", bufs=1, space="PSUM") as sums_pool:
                sq_ps_q = sums_pool.tile([1, TS], F32, tag="sq_q")
                sq_ps_kv = sums_pool.tile([1, TS], F32, tag="sq_kv")
                for mc in range((MT + 1) // 2):
                    ccols = min(256, FUSED - mc * 256)
                    wt = wp.tile([128, KT, 256], F16, tag="wt")
                    nc.sync.dma_start(
                        out=wt[:, :, :ccols],
                        in_=w_fused[:, mc * 256:mc * 256 + ccols].rearrange(
                            "(kt p) m -> p kt m", p=128))
                    for mi in range(2):
                      m = mc * 2 + mi
                      if m >= MT:
                          break
                      mm = min(128, FUSED - m * 128)
                      if True:
                        ps = mp.tile([128, TS], F32, tag="ps")
                        for k in range(KT):
                            nc.tensor.matmul(
                                ps[:mm],
                                wt[:, k, mi * 128:mi * 128 + mm],
                                hidT[:, k, :],
                                start=(k == 0), stop=(k == KT - 1))
                      if m == 16:
                          nc.vector.tensor_copy(qkv16[:mm, m, :], ps[:mm])
                      else:
                          nc.vector.tensor_copy(qkvT[:mm, m, :], ps[:mm])
                      if m < 16:
                          sq = sqp.tile([128, TS], F32R, tag="sq")
                          nc.scalar.square(sq, ps)
                          tgt = sq_ps_q if m < 12 else sq_ps_kv
                          nc.tensor.matmul(tgt, ones_col, sq,
                                           start=(m in (0, 12)),
                                           stop=(m in (11, 15)),
                                           skip_group_check=True)

                # rsqrt(mean(x^2)+eps) = 1/sqrt(sumsq/D + eps)
                rq = small.tile([1, TS], F32, tag="rq")
                nc.scalar.activation(rq, sq_ps_q, func=AF.Sqrt,
                                     scale=1.0 / QLORA, bias=eps_sb)
                nc.vector.reciprocal(rq, rq)
                rq_r = small.tile([1, TS], F32R, tag="rq_r")
                nc.vector.tensor_copy(rq_r, rq)
                rkv = small.tile([1, TS], F32, tag="rkv")
                nc.scalar.activation(rkv, sq_ps_kv, func=AF.Sqrt,
                                     scale=1.0 / KVLORA, bias=eps_sb)
                nc.vector.reciprocal(rkv, rkv)
                rkv_r = small.tile([1, TS], F32R, tag="rkv_r")
                nc.vector.tensor_copy(rkv_r, rkv)
                # broadcast [1,TS] -> [128,TS] via ones-matmul (K=1)
                bq_ps = sums_pool.tile([128, TS], F32, tag="bq")
                nc.tensor.matmul(bq_ps, ones_row, rq_r, start=True, stop=True)
                bkv_ps = sums_pool.tile([128, TS], F32, tag="bkv")
                nc.tensor.matmul(bkv_ps, ones_row, rkv_r, start=True,
                                 stop=True)
                for m in range(16):
                    b = bq_ps if m < 12 else bkv_ps
                    nc.vector.tensor_mul(qkvT[:, m, :], qkvT[:, m, :], b)
                    nc.scalar.activation(qkv16[:, m, :], qkvT[:, m, :],
                                         func=AF.Copy,
                                         scale=ln_sb[:, m:m + 1])

            nc.sync.dma_start(
                out=q_aT_s.rearrange("(mt p) t -> p mt t", p=128),
                in_=qkv16[:, 0:12, :])
            nc.sync.dma_start(
                out=kv_aT_s.rearrange("(mt p) t -> p mt t", p=128),
                in_=qkv16[:, 12:16, :])
            nc.sync.dma_start(out=k_peT_s, in_=qkv16[0:ROPE, 16, :])
    nc.compile()
    return nc


def _build_b():
    nc = bacc.Bacc("TRN2", target_bir_lowering=False, debug=False,
                   num_devices=NCORES)
    q_aT = nc.dram_tensor("q_aT", [QLORA, T], F16, kind="ExternalInput").ap()
    kv_aT = nc.dram_tensor("kv_aT", [KVLORA, T], F16,
                           kind="ExternalInput").ap()
    k_peT = nc.dram_tensor("k_peT", [ROPE, T], F16,
                           kind="ExternalInput").ap()
    w_qb_s = nc.dram_tensor("w_qb_s", [QLORA, HPC * (NOPE + ROPE)], F16,
                            kind="ExternalInput").ap()
    # w_kvb_s host layout: cols = [h0 nope, h1 nope, h0 v, h1 v]
    w_kvb_s = nc.dram_tensor("w_kvb_s", [KVLORA, HPC * (NOPE + VDIM)], F16,
                             kind="ExternalInput").ap()
    w_o_s = nc.dram_tensor("w_o_s", [HPC * VDIM, HID], F16,
                           kind="ExternalInput").ap()
    cos2 = nc.dram_tensor("cos2", [128, T], F16, kind="ExternalInput").ap()
    sin2 = nc.dram_tensor("sin2", [128, T], F16, kind="ExternalInput").ap()
    swap2t = nc.dram_tensor("swap2t", [128, 128], F32R,
                            kind="ExternalInput").ap()
    ident = nc.dram_tensor("ident", [128, 128], F32R,
                           kind="ExternalInput").ap()
    diagm = nc.dram_tensor("diagm", [128, 128], F32, kind="ExternalInput").ap()
    o_part = nc.dram_tensor("o_part", [T, HID], F16,
                            kind="ExternalOutput").ap()

    with tile.TileContext(nc) as tc:
        with tc.tile_pool(name="consts", bufs=1) as consts, \
             tc.tile_pool(name="attn_out", bufs=1) as attn_out:
            ident_sb = consts.tile([128, 128], F32R)
            nc.sync.dma_start(out=ident_sb, in_=ident)
            swap_sb = consts.tile([128, 128], F32R)
            nc.sync.dma_start(out=swap_sb, in_=swap2t)
            swap16 = consts.tile([64, 64], F16)
            nc.vector.tensor_copy(swap16, swap_sb[0:64, 0:64])
            diag_sb = consts.tile([128, 128], F32)
            nc.sync.dma_start(out=diag_sb, in_=diagm)
            ones_row_f32 = consts.tile([1, 128], F32)
            nc.vector.memset(ones_row_f32, 1.0)
            ones_row = consts.tile([1, 128], F32R)
            nc.vector.tensor_copy(ones_row, ones_row_f32)
            zero_f32 = consts.tile([128, 128], F32)
            nc.vector.memset(zero_f32, 0.0)
            zero_sb = consts.tile([128, 128], BF16)
            nc.vector.tensor_copy(zero_sb, zero_f32)
            ident_bf = consts.tile([128, 128], BF16)
            nc.vector.tensor_copy(ident_bf, ident_sb)
            attnT = attn_out.tile([128, HPC, T], F16)

            with tc.tile_pool(name="qk", bufs=1) as qk:
                qn_t = qk.tile([128, HPC, T], F32R)   # q nope, per head
                qpe_ro = qk.tile([128, T], F32R)
                qpe_ro_h1 = qk.tile([64, T], F32R)    # h1 rows rebased to p0
                kn_t = qk.tile([128, HPC, T], F32R)   # k nope, per head
                kpe_ro = qk.tile([64, T], F32R)
                vt = qk.tile([128, TT, HPC * VDIM], BF16)  # v token-major
                tmpq_ctx = tc.tile_pool(name="tmpq", bufs=1)
                tmpq = tmpq_ctx.__enter__()
                qpe = tmpq.tile([128, T], F32R)       # q pe stacked h0|h1
                kpe_raw = tmpq.tile([64, T], F16)
                nc.sync.dma_start(out=kpe_raw, in_=k_peT)

                # ---- kv_b projection -> feature-major k_nope + token-major v
                with tc.tile_pool(name="kva_p", bufs=2) as kva_p, \
                     tc.tile_pool(name="wkvb_p", bufs=1) as wkvb_p, \
                     tc.tile_pool(name="kvpsum", bufs=3, space="PSUM") as kvps:
                    wkvb = wkvb_p.tile([128, 4, 512], F16)
                    nc.sync.dma_start(
                        out=wkvb,
                        in_=w_kvb_s.rearrange("(kt p) m -> p kt m", p=128))
                    for n in range(NCH):
                        ncol = slice(n * 512, (n + 1) * 512)
                        kva_n = kva_p.tile([128, 4, 512], F16, tag="kva")
                        for k in range(4):
                            nc.sync.dma_start(
                                out=kva_n[:, k, :],
                                in_=kv_aT[k * 128:(k + 1) * 128, ncol])
                        for h in range(HPC):
                            ps = kvps.tile([128, 512], F32, tag="knmm")
                            for k in range(4):
                                nc.tensor.matmul(
                                    ps, wkvb[:, k, h * 128:(h + 1) * 128],
                                    kva_n[:, k, :],
                                    start=(k == 0), stop=(k == 3))
                            nc.scalar.copy(kn_t[:, h, ncol], ps)
                        for stl in range(4):
                            st = n * 4 + stl
                            ps = kvps.tile([128, 256], F32, tag="vmm")
                            for k in range(4):
                                nc.tensor.matmul(
                                    ps,
                                    kva_n[:, k, stl * 128:(stl + 1) * 128],
                                    wkvb[:, k, 256:512],
                                    start=(k == 0), stop=(k == 3))
                            nc.scalar.copy(vt[:, st, :], ps)

                # ---- q_b projection -> feature-major q, with rope
                #      interleaved per 512-token chunk
                with tc.tile_pool(name="qa_p", bufs=2) as qa_p, \
                     tc.tile_pool(name="wqb_p", bufs=1) as wqb_p, \
                     tc.tile_pool(name="ropec", bufs=1) as ropec, \
                     tc.tile_pool(name="rps", bufs=2, space="PSUM") as rps, \
                     tc.tile_pool(name="qpsum", bufs=3, space="PSUM") as qps:
                    cos_sb = ropec.tile([128, T], F16)
                    nc.sync.dma_start(out=cos_sb, in_=cos2)
                    sin_sb = ropec.tile([128, T], F16)
                    nc.sync.dma_start(out=sin_sb, in_=sin2)
                    # k_pe rope up front (independent of projections)
                    for n in range(NCH):
                        ncol = slice(n * 512, (n + 1) * 512)
                        ps2 = rps.tile([64, 512], F32, tag="swk")
                        nc.tensor.matmul(ps2, swap16,
                                         kpe_raw[:, ncol],
                                         start=True, stop=True)
                        nc.vector.tensor_mul(kpe_ro[:, ncol],
                                             kpe_raw[:, ncol],
                                             cos_sb[0:64, ncol])
                        nc.vector.tensor_mul(ps2, ps2, sin_sb[0:64, ncol])
                        nc.vector.tensor_add(kpe_ro[:, ncol], kpe_ro[:, ncol],
                                             ps2)
                    wqb = wqb_p.tile([128, 12, 384], F16)
                    nc.sync.dma_start(
                        out=wqb,
                        in_=w_qb_s.rearrange("(kt p) m -> p kt m", p=128))
                    for n in range(NCH):
                        ncol = slice(n * 512, (n + 1) * 512)
                        qa_n = qa_p.tile([128, 12, 512], F16, tag="qa")
                        for k in range(12):
                            nc.sync.dma_start(
                                out=qa_n[:, k, :],
                                in_=q_aT[k * 128:(k + 1) * 128, ncol])
                        for m in range(3):
                            ps = qps.tile([128, 512], F32, tag="qmm")
                            for k in range(12):
                                nc.tensor.matmul(
                                    ps, wqb[:, k, m * 128:(m + 1) * 128],
                                    qa_n[:, k, :],
                                    start=(k == 0), stop=(k == 11))
                            if m == 0:
                                nc.scalar.copy(qn_t[:, 0, ncol], ps)
                            elif m == 1:
                                nc.scalar.copy(qpe[0:64, ncol], ps[0:64])
                                nc.scalar.copy(qn_t[0:64, 1, ncol],
                                               ps[64:128])
                            else:
                                nc.scalar.copy(qn_t[64:128, 1, ncol],
                                               ps[0:64])
                                nc.scalar.copy(qpe[64:128, ncol],
                                               ps[64:128])
                        # rope for this chunk
                        psr = rps.tile([128, 512], F32, tag="swq")
                        nc.tensor.matmul(psr, swap_sb, qpe[:, ncol],
                                         start=True, stop=True)
                        nc.vector.tensor_mul(qpe_ro[:, ncol], qpe[:, ncol],
                                             cos_sb[:, ncol])
                        nc.vector.tensor_mul(psr, psr, sin_sb[:, ncol])
                        nc.vector.tensor_add(qpe_ro[:, ncol], qpe_ro[:, ncol],
                                             psr)
                        nc.vector.tensor_copy(qpe_ro_h1[:, ncol],
                                              qpe_ro[64:128, ncol])

                tmpq_ctx.__exit__(None, None, None)
                # ---- causal attention (2 heads) interleaved with o_proj,
                #      chunk by chunk (512 tokens)
                with tc.tile_pool(name="pT", bufs=1) as ptp, \
                     tc.tile_pool(name="prb", bufs=8) as prp, \
                     tc.tile_pool(name="smp", bufs=6) as smp, \
                     tc.tile_pool(name="osb_p", bufs=6) as osb_p, \
                     tc.tile_pool(name="wo_p", bufs=1) as wo_p, \
                     tc.tile_pool(name="scps", bufs=3, space="PSUM") as scps, \
                     tc.tile_pool(name="tps", bufs=2, space="PSUM") as tps, \
                     tc.tile_pool(name="pvps", bufs=1, space="PSUM") as pvps, \
                     tc.tile_pool(name="ops", bufs=2, space="PSUM") as ops:
                    wo = wo_p.tile([128, HPC, HID], F16)
                    nc.sync.dma_start(
                        out=wo, in_=w_o_s.rearrange("(kt p) m -> p kt m", p=128))
                    probst = ptp.tile([128, TT, 512], BF16)
                    for c in range(NCH):
                        for h in range(HPC):
                            for tl in range(4):
                                tt = c * 4 + tl
                                tcol = slice(tt * 128, (tt + 1) * 128)
                                nsc = tt // 4 + 1
                                sums = smp.tile([128, 4], F32, tag="sums")
                                prs = []
                                for sc in range(nsc):
                                    scol = slice(sc * 512, (sc + 1) * 512)
                                    ps = scps.tile([128, 512], F32, tag="sc")
                                    nc.tensor.matmul(ps, qn_t[:, h, tcol],
                                                     kn_t[:, h, scol],
                                                     start=True, stop=False)
                                    qpe_l = (qpe_ro[0:64, tcol] if h == 0
                                             else qpe_ro_h1[:, tcol])
                                    nc.tensor.matmul(ps, qpe_l,
                                                     kpe_ro[:, scol],
                                                     start=False, stop=True)
                                    if sc == tt // 4:
                                        d = tt * 128 - sc * 512
                                        nc.vector.tensor_add(
                                            ps[:, d:d + 128],
                                            ps[:, d:d + 128], diag_sb)
                                        if d + 128 < 512:
                                            nc.vector.memset(
                                                ps[:, d + 128:512], NEG)
                                    pr = prp.tile([128, 512], BF16, tag="pr")
                                    nc.scalar.activation(
                                        pr, ps, func=AF.Exp, scale=SCALING,
                                        accum_out=sums[:, sc:sc + 1])
                                    prs.append((sc, pr))
                                rt = smp.tile([128, 1], F32, tag="rt")
                                nc.vector.reduce_sum(rt, sums[:, 0:nsc],
                                                     axis=AX.X)
                                nc.vector.reciprocal(rt, rt)
                                for sc, pr in prs:
                                    nc.vector.tensor_scalar_mul(pr, pr, rt)
                                    for b in range(4):
                                        st = sc * 4 + b
                                        dst = probst[:, st,
                                                     tl * 128:(tl + 1) * 128]
                                        if st > tt:
                                            # strictly-future block: all zeros
                                            nc.vector.tensor_copy(dst, zero_sb)
                                            continue
                                        ps2 = tps.tile([128, 128], BF16,
                                                       tag="tr")
                                        nc.tensor.transpose(
                                            ps2,
                                            pr[:, b * 128:(b + 1) * 128],
                                            ident_bf)
                                        nc.vector.tensor_copy(dst, ps2)
                            # PV
                            pv = pvps.tile([128, 512], F32, tag="pv")
                            ns_t = 4 * (c + 1)
                            for st in range(ns_t):
                                nc.tensor.matmul(
                                    pv, vt[:, st, h * 128:(h + 1) * 128],
                                    probst[:, st, :],
                                    start=(st == 0), stop=(st == ns_t - 1))
                            acol = slice(c * 512, (c + 1) * 512)
                            nc.vector.tensor_copy(attnT[:, h, acol], pv)
                        # o_proj for this chunk's 4 token tiles (RowParallel)
                        for tl in range(4):
                            tt = c * 4 + tl
                            tcol = slice(tt * 128, (tt + 1) * 128)
                            for nch in range(HID // 512):
                                ps = ops.tile([128, 512], F32, tag="op")
                                for h in range(HPC):
                                    nc.tensor.matmul(
                                        ps, attnT[:, h, tcol],
                                        wo[:, h, nch * 512:(nch + 1) * 512],
                                        start=(h == 0), stop=(h == HPC - 1))
                                osb = osb_p.tile([128, 512], F16, tag="osb")
                                if nch % 5 in (0, 2):
                                    nc.vector.tensor_copy(osb, ps)
                                else:
                                    nc.scalar.copy(osb, ps)
                                nc.sync.dma_start(
                                    out=o_part[tt * 128:(tt + 1) * 128,
                                               nch * 512:(nch + 1) * 512],
                                    in_=osb)
    nc.compile()
    return nc


_CACHE = {}


def _get(name):
    if name not in _CACHE:
        _CACHE[name] = _build_a() if name == "a" else _build_b()
    return _CACHE[name]


def _host_consts():
    ident = np.eye(128, dtype=np.float32)
    # swap matrix S: (Sx)[2i] = -x[2i+1], (Sx)[2i+1] = x[2i]; we pass S^T,
    # block-diag over the two 64-row head slots.
    st64 = np.zeros((64, 64), dtype=np.float32)
    for i in range(32):
        st64[2 * i, 2 * i + 1] = 1.0
        st64[2 * i + 1, 2 * i] = -1.0
    swap2t = np.zeros((128, 128), dtype=np.float32)
    swap2t[0:64, 0:64] = st64
    swap2t[64:128, 64:128] = st64
    r = np.arange(128)
    diagm = np.where(r[None, :] <= r[:, None], 0.0, NEG).astype(np.float32)
    return ident, swap2t, diagm


def _rope_tables(positions):
    # duplicated-pair (interleaved) layout, rows stacked twice for 2 heads
    inv_freq = 1.0 / (THETA ** (np.arange(0, ROPE, 2, dtype=np.float32)
                                / ROPE))
    freqs = positions.astype(np.float32)[:, None] * inv_freq[None, :]  # [T,32]
    cos = np.cos(freqs).astype(np.float32)
    sin = np.sin(freqs).astype(np.float32)
    cos_dup = np.repeat(cos, 2, axis=1).T.copy()   # [64, T]
    sin_dup = np.repeat(sin, 2, axis=1).T.copy()
    cos2 = np.vstack([cos_dup, cos_dup])           # [128, T]
    sin2 = np.vstack([sin_dup, sin_dup])
    return np.ascontiguousarray(cos2), np.ascontiguousarray(sin2)


def kernel(positions, hidden_states, w_fused, q_a_ln_w, kv_a_ln_w,
           w_qb, w_kvb, w_o):
    positions = np.asarray(positions)
    hidden_states = np.ascontiguousarray(np.asarray(hidden_states,
                                                    dtype=np.float32))
    w_fused = np.ascontiguousarray(np.asarray(w_fused, dtype=np.float32))
    q_a_ln_w = np.ascontiguousarray(np.asarray(q_a_ln_w, dtype=np.float32))
    kv_a_ln_w = np.ascontiguousarray(np.asarray(kv_a_ln_w, dtype=np.float32))
    w_qb = np.asarray(w_qb, dtype=np.float32)
    w_kvb = np.asarray(w_kvb, dtype=np.float32)
    w_o = np.asarray(w_o, dtype=np.float32)

    ident, swap2t, diagm = _host_consts()
    w_fused16 = w_fused.astype(np.float16)
    cos2, sin2 = _rope_tables(positions)
    cos2 = cos2.astype(np.float16)
    sin2 = sin2.astype(np.float16)

    # ---- launch A: sequence-parallel fused projection + norms
    nca = _get("a")
    in_a = []
    for c in range(NCORES):
        in_a.append({
            "hid_s": np.ascontiguousarray(
                hidden_states[c * TS:(c + 1) * TS, :]).astype(np.float16),
            "w_fused": w_fused16,
            "q_ln": q_a_ln_w,
            "kv_ln": kv_a_ln_w,
            "ident": ident,
        })
    res_a = bass_utils.run_bass_kernel_spmd(nca, in_a,
                                            core_ids=list(range(NCORES)))
    q_aT = np.concatenate([res_a.results[c]["q_aT_s"]
                           for c in range(NCORES)], axis=1)
    kv_aT = np.concatenate([res_a.results[c]["kv_aT_s"]
                            for c in range(NCORES)], axis=1)
    k_peT = np.concatenate([res_a.results[c]["k_peT_s"]
                            for c in range(NCORES)], axis=1)

    # ---- launch B: head-parallel attention
    ncb = _get("b")
    in_b = []
    for c in range(NCORES):
        g0, g1 = 2 * c, 2 * c + 1
        wq_s = np.ascontiguousarray(
            w_qb[:, g0 * (NOPE + ROPE):(g1 + 1) * (NOPE + ROPE)]
        ).astype(np.float16)
        wk = w_kvb
        wkv_s = np.ascontiguousarray(np.concatenate([
            wk[:, g0 * 256:g0 * 256 + 128],        # h0 nope
            wk[:, g1 * 256:g1 * 256 + 128],        # h1 nope
            wk[:, g0 * 256 + 128:(g0 + 1) * 256],  # h0 v
            wk[:, g1 * 256 + 128:(g1 + 1) * 256],  # h1 v
        ], axis=1)).astype(np.float16)
        wo_s = np.ascontiguousarray(
            w_o[g0 * VDIM:(g1 + 1) * VDIM, :]).astype(np.float16)
        in_b.append({
            "q_aT": q_aT, "kv_aT": kv_aT, "k_peT": k_peT,
            "w_qb_s": wq_s, "w_kvb_s": wkv_s, "w_o_s": wo_s,
            "cos2": cos2, "sin2": sin2,
            "swap2t": swap2t, "ident": ident, "diagm": diagm,
        })
    res_b = bass_utils.run_bass_kernel_spmd(ncb, in_b,
                                            core_ids=list(range(NCORES)))
    out = res_b.results[0]["o_part"].astype(np.float64)
    for c in range(1, NCORES):
        out += res_b.results[c]["o_part"]
    return out.astype(np.float32)



# revision 2
# speedup vs baseline: 1.0004x; 1.0004x over previous
"""DeepseekV2 MLA prefill kernel for 8 Trainium2 NeuronCores (v2).

Launch A: sequence-parallel fused qkv_a projection + RMSNorms.
  Host pre-transposes each core's 256-token hidden slice to [HID, TS]
  so no on-device transposes are needed. RMSNorm is applied as a
  rank-1 outer-product scale (ln[m] x rsqrt[t]) computed on the PE,
  then one DVE multiply per m-tile, overlapped with remaining matmuls.

Launch B: tensor-parallel over heads (2 heads/core).
  Scores are computed TRANSPOSED ([keys, q]) so that:
    - softmax normalization folds into PV: the moving V operand gets an
      extra ones-column, so PV emits [attn | denom] per q-tile and the
      normalization is a per-partition scalar multiply.
    - no per-block probability transposes / rescaling are needed.
  attn is transposed once per q-tile (128 rows) for the RowParallel
  o_proj. All DMAs are batched (one per logical tensor / chunk);
  o_part is stored as 16 row-tile DMAs of [128, 5120].
"""

import numpy as np
import concourse.bacc as bacc
import concourse.tile as tile
from concourse import mybir
from concourse import bass_utils

F32 = mybir.dt.float32
F32R = mybir.dt.float32r
F16 = mybir.dt.float16
AF = mybir.ActivationFunctionType
AX = mybir.AxisListType

NCORES = 8
T, HID, H = 2048, 5120, 16
NOPE, ROPE, VDIM = 128, 64, 128
QLORA, KVLORA = 1536, 512
FUSED = QLORA + KVLORA + ROPE  # 2112
TS = T // NCORES               # 256 tokens/core in launch A
HPC = H // NCORES              # 2 heads/core in launch B
EPS = 1e-6
THETA = 10000.0
SCALING = float((NOPE + ROPE) ** -0.5)
NEG = -1.0e30
KT = HID // 128                # 40
MT = (FUSED + 127) // 128      # 17 (last tile is 64 rows)
TT = T // 128                  # 16
NCH = T // 512                 # 4
EXPB = -4.0                    # uniform exp bias (cancels in softmax)


def _build_a():
    nc = bacc.Bacc("TRN2", target_bir_lowering=False, debug=False,
                   num_devices=NCORES)
    hid_sT = nc.dram_tensor("hid_sT", [HID, TS], F16,
                            kind="ExternalInput").ap()
    w_fused = nc.dram_tensor("w_fused", [HID, FUSED], F16,
                             kind="ExternalInput").ap()
    q_ln = nc.dram_tensor("q_ln", [QLORA], F32, kind="ExternalInput").ap()
    kv_ln = nc.dram_tensor("kv_ln", [KVLORA], F32, kind="ExternalInput").ap()
    q_aT_s = nc.dram_tensor("q_aT_s", [QLORA, TS], F16,
                            kind="ExternalOutput").ap()
    kv_aT_s = nc.dram_tensor("kv_aT_s", [KVLORA, TS], F16,
                             kind="ExternalOutput").ap()
    k_peT_s = nc.dram_tensor("k_peT_s", [ROPE, TS], F16,
                             kind="ExternalOutput").ap()

    with tile.TileContext(nc) as tc:
        with tc.tile_pool(name="consts", bufs=1) as consts, \
             tc.tile_pool(name="hidT_pool", bufs=1) as hidT_pool, \
             tc.tile_pool(name="qkv_pool", bufs=1) as qkv_pool, \
             tc.tile_pool(name="small", bufs=1) as small:
            # hidT[p, k, t] = hidden_slice[t, k*128+p]; host pre-transposed.
            hidT = hidT_pool.tile([128, KT, TS], F16)
            nc.sync.dma_start(
                out=hidT[:, 0:10, :],
                in_=hid_sT[0:10 * 128, :].rearrange("(kt p) t -> p kt t",
                                                    p=128))
            # ln row: [1, 2048] = [q_ln | kv_ln] as f32r for scale matmuls
            ln_f32 = consts.tile([1, QLORA + KVLORA], F32)
            ln_row = consts.tile([1, QLORA + KVLORA], F32R)
            ones_f32 = consts.tile([128, 1], F32)
            ones_col = consts.tile([128, 1], F32R)
            eps_sb = small.tile([1, 1], F32)

            qkv16 = qkv_pool.tile([128, MT, TS], F16)
            with tc.tile_pool(name="wpool", bufs=4) as wp, \
                 tc.tile_pool(name="mpsum", bufs=3, space="PSUM") as mp, \
                 tc.tile_pool(name="sqpool", bufs=3) as sqp, \
                 tc.tile_pool(name="sumps", bufs=1, space="PSUM") as sums_pool, \
                 tc.tile_pool(name="sclps", bufs=3, space="PSUM") as scl_pool:
                sq_ps_q = sums_pool.tile([1, TS], F32, tag="sq_q")
                sq_ps_kv = sums_pool.tile([1, TS], F32, tag="sq_kv")

                # weight chunks: 7 of 256 cols + 1 of 320 cols (m14..16)
                wts = []
                for mc in range(8):
                    ccols = 256 if mc < 7 else 320
                    wt = wp.tile([128, KT, 320], F16, tag="wt")
                    if mc == 0:
                        # split first chunk by k to start compute earlier
                        nc.sync.dma_start(
                            out=wt[:, 0:10, :ccols],
                            in_=w_fused[0:1280, 0:ccols].rearrange(
                                "(kt p) m -> p kt m", p=128))
                        nc.sync.dma_start(
                            out=hidT[:, 10:20, :],
                            in_=hid_sT[1280:2560, :].rearrange(
                                "(kt p) t -> p kt t", p=128))
                        nc.sync.dma_start(
                            out=wt[:, 10:20, :ccols],
                            in_=w_fused[1280:2560, 0:ccols].rearrange(
                                "(kt p) m -> p kt m", p=128))
                        nc.sync.dma_start(
                            out=hidT[:, 20:40, :],
                            in_=hid_sT[2560:5120, :].rearrange(
                                "(kt p) t -> p kt t", p=128))
                        nc.sync.dma_start(
                            out=wt[:, 20:40, :ccols],
                            in_=w_fused[2560:5120, 0:ccols].rearrange(
                                "(kt p) m -> p kt m", p=128))
                    else:
                        nc.sync.dma_start(
                            out=wt[:, :, :ccols],
                            in_=w_fused[:, mc * 256:mc * 256 + ccols].rearrange(
                                "(kt p) m -> p kt m", p=128))
                    if mc == 1:
                        nc.sync.dma_start(
                            out=ln_f32[:, 0:QLORA],
                            in_=q_ln.rearrange("(o a) -> o a", o=1))
                        nc.sync.dma_start(
                            out=ln_f32[:, QLORA:],
                            in_=kv_ln.rearrange("(o a) -> o a", o=1))
                        nc.vector.tensor_copy(ln_row, ln_f32)
                        nc.vector.memset(ones_f32, 1.0)
                        nc.vector.tensor_copy(ones_col, ones_f32)
                        nc.vector.memset(eps_sb, EPS)
                    wts.append(wt)

                rq_r = small.tile([1, TS], F32R, tag="rq_r")
                rkv_r = small.tile([1, TS], F32R, tag="rkv_r")

                def norm_scale(m, r_vec):
                    # scale_ps[p, t] = ln[m*128+p] * r_vec[t]  (rank-1 matmul)
                    sp = scl_pool.tile([128, TS], F32, tag="scl")
                    nc.tensor.matmul(sp, ln_row[:, m * 128:(m + 1) * 128],
                                     r_vec, start=True, stop=True)
                    nc.vector.tensor_mul(qkv16[:, m, :], qkv16[:, m, :], sp)

                ps01 = None
                pend_sq = []
                for m in range(MT):
                    mm = min(128, FUSED - m * 128)
                    wt = wts[min(m // 2, 7)]
                    mi = m % 2 if m < 14 else m - 14
                    if m == 0:
                        # first chunk: interleave m0/m1 chains per k-group so
                        # the PE tracks the hidT/wt0 DMA stream
                        ps01 = [mp.tile([128, TS], F32, tag="ps",
                                        name=f"ps0{i}") for i in range(2)]
                        for k0, k1 in ((0, 5), (5, 10), (10, 20), (20, 40)):
                            for i in range(2):
                                for k in range(k0, k1):
                                    nc.tensor.matmul(
                                        ps01[i],
                                        wt[:, k, i * 128:(i + 1) * 128],
                                        hidT[:, k, :],
                                        start=(k == 0), stop=(k == KT - 1),
                                        skip_group_check=True)
                        ps = ps01[0]
                    elif m == 1:
                        ps = ps01[1]
                    else:
                        ps = mp.tile([128, TS], F32, tag="ps")
                        for k in range(KT):
                            nc.tensor.matmul(
                                ps[:mm],
                                wt[:, k, mi * 128:mi * 128 + mm],
                                hidT[:, k, :],
                                start=(k == 0), stop=(k == KT - 1))
                    nc.vector.tensor_copy(qkv16[:mm, m, :], ps[:mm])
                    if m < 16:
                        sq = sqp.tile([128, TS], F32R, tag="sq")
                        nc.scalar.square(sq, ps)
                        pend_sq.append((m, sq))
                    # flush deferred sumsq matmuls (keep PE off the Act
                    # critical path); force flush at the boundaries
                    while pend_sq and (pend_sq[0][0] < m or m in (12, 16)):
                        pm, psq = pend_sq.pop(0)
                        nc.tensor.matmul(sq_ps_q if pm < 12 else sq_ps_kv,
                                         ones_col, psq,
                                         start=(pm in (0, 12)),
                                         stop=(pm in (11, 15)),
                                         skip_group_check=True)
                    if m == 12:
                        # rsqrt(mean(x^2)+eps) = 1/sqrt(sumsq/D + eps)
                        rq = small.tile([1, TS], F32, tag="rq")
                        nc.scalar.activation(rq, sq_ps_q, func=AF.Sqrt,
                                             scale=1.0 / QLORA, bias=eps_sb)
                        nc.vector.reciprocal(rq, rq)
                        nc.vector.tensor_copy(rq_r, rq)
                    if m == 13:
                        for mm2 in range(0, 6):
                            norm_scale(mm2, rq_r)
                    if m == 14:
                        for mm2 in range(6, 12):
                            norm_scale(mm2, rq_r)
                    if m == 15:
                        nc.sync.dma_start(
                            out=q_aT_s.rearrange("(mt p) t -> p mt t", p=128),
                            in_=qkv16[:, 0:12, :])
                    if m == 16:
                        rkv = small.tile([1, TS], F32, tag="rkv")
                        nc.scalar.activation(rkv, sq_ps_kv, func=AF.Sqrt,
                                             scale=1.0 / KVLORA, bias=eps_sb)
                        nc.vector.reciprocal(rkv, rkv)
                        nc.vector.tensor_copy(rkv_r, rkv)
                for mm2 in range(12, 16):
                    norm_scale(mm2, rkv_r)
                nc.sync.dma_start(
                    out=kv_aT_s.rearrange("(mt p) t -> p mt t", p=128),
                    in_=qkv16[:, 12:16, :])
                nc.sync.dma_start(out=k_peT_s, in_=qkv16[0:ROPE, 16, :])
    nc.compile()
    return nc


def _build_b():
    nc = bacc.Bacc("TRN2", target_bir_lowering=False, debug=False,
                   num_devices=NCORES)
    q_aT = nc.dram_tensor("q_aT", [QLORA, T], F16, kind="ExternalInput").ap()
    kv_aT = nc.dram_tensor("kv_aT", [KVLORA, T], F16,
                           kind="ExternalInput").ap()
    k_peT = nc.dram_tensor("k_peT", [ROPE, T], F16,
                           kind="ExternalInput").ap()
    w_qb_s = nc.dram_tensor("w_qb_s", [QLORA, HPC * (NOPE + ROPE)], F16,
                            kind="ExternalInput").ap()
    # w_kvb_s host layout: cols = [h0 nope, h1 nope, h0 v, h1 v]
    w_kvb_s = nc.dram_tensor("w_kvb_s", [KVLORA, HPC * (NOPE + VDIM)], F16,
                             kind="ExternalInput").ap()
    w_o_s = nc.dram_tensor("w_o_s", [HPC * VDIM, HID], F16,
                           kind="ExternalInput").ap()
    cos2 = nc.dram_tensor("cos2", [128, T], F16, kind="ExternalInput").ap()
    sin2 = nc.dram_tensor("sin2", [128, T], F16, kind="ExternalInput").ap()
    swap2t = nc.dram_tensor("swap2t", [128, 128], F16,
                            kind="ExternalInput").ap()
    ident = nc.dram_tensor("ident", [128, 128], F16,
                           kind="ExternalInput").ap()
    # diagT[r, c] = NEG where key r > query c (transposed causal mask)
    diagT = nc.dram_tensor("diagT", [128, 128], F32, kind="ExternalInput").ap()
    o_part = nc.dram_tensor("o_part", [T, HID], F16,
                            kind="ExternalOutput").ap()

    with tile.TileContext(nc) as tc:
        with tc.tile_pool(name="consts", bufs=1) as consts, \
             tc.tile_pool(name="persist", bufs=1) as persist:
            # ---- batched input DMAs (ordered for pipeline) ----
            wkvb = persist.tile([128, 4, 512], F16)
            nc.sync.dma_start(
                out=wkvb, in_=w_kvb_s.rearrange("(kt p) m -> p kt m", p=128))
            kva_pool = tc.alloc_tile_pool(name="kva_pool", bufs=2)
            qa_pool = tc.alloc_tile_pool(name="qa_pool", bufs=2)
            kva_ch = [kva_pool.tile([128, 4, 512], F16, tag="kva",
                                    name=f"kva{n}") for n in range(NCH)]
            qa_ch = [qa_pool.tile([128, 12, 512], F16, tag="qa",
                                  name=f"qa{n}") for n in range(NCH)]
            nc.sync.dma_start(
                out=kva_ch[0][:, 0:2, :],
                in_=kv_aT[0:256, 0:512].rearrange("(kt p) t -> p kt t",
                                                  p=128))
            nc.sync.dma_start(
                out=kva_ch[0][:, 2:4, :],
                in_=kv_aT[256:512, 0:512].rearrange("(kt p) t -> p kt t",
                                                    p=128))
            nc.sync.dma_start(
                out=kva_ch[1],
                in_=kv_aT[:, 512:1024].rearrange("(kt p) t -> p kt t", p=128))
            wqb = persist.tile([128, 12, 384], F16)
            nc.sync.dma_start(
                out=wqb, in_=w_qb_s.rearrange("(kt p) m -> p kt m", p=128))
            nc.sync.dma_start(
                out=qa_ch[0][:, 0:6, :],
                in_=q_aT[0:768, 0:512].rearrange("(kt p) t -> p kt t", p=128))
            nc.sync.dma_start(
                out=qa_ch[0][:, 6:12, :],
                in_=q_aT[768:1536, 0:512].rearrange("(kt p) t -> p kt t",
                                                    p=128))
            cos_sb = persist.tile([128, T], F16)
            nc.sync.dma_start(out=cos_sb, in_=cos2)
            sin_sb = persist.tile([128, T], F16)
            nc.sync.dma_start(out=sin_sb, in_=sin2)
            kpe_raw = persist.tile([64, T], F16)
            nc.sync.dma_start(out=kpe_raw, in_=k_peT)
            swap_sb = consts.tile([128, 128], F16)
            nc.sync.dma_start(out=swap_sb, in_=swap2t)
            ident_sb = consts.tile([128, 128], F16)
            nc.sync.dma_start(out=ident_sb, in_=ident)
            diag_sb = consts.tile([128, 128], F32)
            nc.sync.dma_start(out=diag_sb, in_=diagT)
            expb_sb = consts.tile([128, 1], F32)
            nc.vector.memset(expb_sb, EXPB)
            for n in range(2, NCH):
                ncol = slice(n * 512, (n + 1) * 512)
                nc.sync.dma_start(
                    out=kva_ch[n],
                    in_=kv_aT[:, ncol].rearrange("(kt p) t -> p kt t", p=128))
            for n in range(1, NCH):
                ncol = slice(n * 512, (n + 1) * 512)
                nc.sync.dma_start(
                    out=qa_ch[n],
                    in_=q_aT[:, ncol].rearrange("(kt p) t -> p kt t", p=128))
            wo = persist.tile([128, HPC, HID], F16)
            nc.sync.dma_start(
                out=wo, in_=w_o_s.rearrange("(kt p) m -> p kt m", p=128))

            # ---- persistent activations (per-chunk tiles so consumers
            # depend only on their own chunk's writes) ----
            kn_c = [persist.tile([128, HPC, 512], F16, name=f"kn{n}")
                    for n in range(NCH)]
            kpe_c = [persist.tile([64, 512], F16, name=f"kpe{n}")
                     for n in range(NCH)]
            qn_c = [persist.tile([128, HPC, 512], F16, name=f"qn{n}")
                    for n in range(NCH)]
            qpe_c = [persist.tile([128, 512], F16, name=f"qpe{n}")
                     for n in range(NCH)]
            qpe1_c = [persist.tile([64, 512], F16, name=f"qpe1_{n}")
                      for n in range(NCH)]
            # v token-major with ones column: [token-in-tile, st, h, 129]
            vt_c = [persist.tile([128, 4, HPC, NOPE + 1], F16,
                                 name=f"vt{n}") for n in range(NCH)]
            for n in range(NCH):
                nc.vector.memset(vt_c[n][:, :, :, NOPE:NOPE + 1], 1.0)

            # ---- projections + rope, chunk by chunk ----
            with tc.tile_pool(name="qpe_p", bufs=1) as qpe_p, \
                 tc.tile_pool(name="prps", bufs=4, space="PSUM") as prps, \
                 tc.tile_pool(name="rps", bufs=2, space="PSUM") as rps:
                qpe = qpe_p.tile([128, T], F16)  # unroped q_pe h0|h1

                def emit_kvb(n):
                    kva = kva_ch[n]
                    # k_nope per head (feature-major out)
                    for h in range(HPC):
                        ps = prps.tile([128, 512], F32, tag="pp")
                        for k in range(4):
                            nc.tensor.matmul(
                                ps, wkvb[:, k, h * 128:(h + 1) * 128],
                                kva[:, k, :],
                                start=(k == 0), stop=(k == 3))
                        nc.scalar.copy(kn_c[n][:, h, :], ps)
                    # v token-major (tokens on partitions)
                    for stl in range(4):
                        tcol = slice(stl * 128, (stl + 1) * 128)
                        ps = prps.tile([128, 512], F32, tag="pp")
                        for k in range(4):
                            nc.tensor.matmul(
                                ps[:, 0:256],
                                kva[:, k, tcol],
                                wkvb[:, k, 256:512],
                                start=(k == 0), stop=(k == 3))
                        for h in range(HPC):
                            if h == 0:
                                nc.vector.tensor_copy(
                                    vt_c[n][:, stl, h, 0:NOPE],
                                    ps[:, h * 128:(h + 1) * 128])
                            else:
                                nc.scalar.copy(vt_c[n][:, stl, h, 0:NOPE],
                                               ps[:, h * 128:(h + 1) * 128])

                def emit_qb_rope(n):
                    ncol = slice(n * 512, (n + 1) * 512)
                    qa = qa_ch[n]
                    # q_b -> feature-major q (3 m-tiles of 128)
                    for m in range(3):
                        ps = prps.tile([128, 512], F32, tag="pp")
                        for k in range(12):
                            nc.tensor.matmul(
                                ps, wqb[:, k, m * 128:(m + 1) * 128],
                                qa[:, k, :],
                                start=(k == 0), stop=(k == 11))
                        if m == 0:
                            nc.scalar.copy(qn_c[n][:, 0, :], ps)
                        elif m == 1:
                            nc.scalar.copy(qpe[0:64, ncol], ps[0:64])
                            nc.scalar.copy(qn_c[n][0:64, 1, :], ps[64:128])
                        else:
                            nc.scalar.copy(qn_c[n][64:128, 1, :], ps[0:64])
                            nc.scalar.copy(qpe[64:128, ncol], ps[64:128])
                    # rope for this chunk: q_pe (both heads) and k_pe
                    psr = rps.tile([128, 512], F32, tag="swq")
                    nc.tensor.matmul(psr, swap_sb, qpe[:, ncol],
                                     start=True, stop=True)
                    nc.vector.tensor_mul(qpe_c[n], qpe[:, ncol],
                                         cos_sb[:, ncol])
                    nc.vector.tensor_mul(psr, psr, sin_sb[:, ncol])
                    nc.vector.tensor_add(qpe_c[n], qpe_c[n], psr)
                    nc.vector.tensor_copy(qpe1_c[n], qpe_c[n][64:128, :])
                    ps2 = rps.tile([64, 512], F32, tag="swk")
                    nc.tensor.matmul(ps2, swap_sb[0:64, 0:64],
                                     kpe_raw[:, ncol], start=True, stop=True)
                    nc.vector.tensor_mul(kpe_c[n], kpe_raw[:, ncol],
                                         cos_sb[0:64, ncol])
                    nc.vector.tensor_mul(ps2, ps2, sin_sb[0:64, ncol])
                    nc.vector.tensor_add(kpe_c[n], kpe_c[n], ps2)

                # kvb c0+c1 run while wqb / qa c0 stream in
                emit_kvb(0)
                emit_kvb(1)
                emit_qb_rope(0)
                emit_kvb(2)
                emit_qb_rope(1)
                emit_kvb(3)
                emit_qb_rope(2)
                emit_qb_rope(3)
            qa_pool.release()
            kva_pool.release()

            # ---- attention (transposed scores) + o_proj ----
            # o_proj of chunk c-1 is interleaved with the scores matmuls
            # of chunk c so the PE stays busy while Act drains the exps.
            with tc.tile_pool(name="expp", bufs=2) as expp, \
                 tc.tile_pool(name="attn_p", bufs=2) as attn_p, \
                 tc.tile_pool(name="asb_p", bufs=4) as asb_p, \
                 tc.tile_pool(name="rcp_p", bufs=4) as rcp_p, \
                 tc.tile_pool(name="osb_p", bufs=4) as osb_p, \
                 tc.tile_pool(name="scps", bufs=2, space="PSUM") as scps, \
                 tc.tile_pool(name="pvps", bufs=2, space="PSUM") as pvps, \
                 tc.tile_pool(name="trps", bufs=1, space="PSUM") as trps, \
                 tc.tile_pool(name="ops", bufs=3, space="PSUM") as ops:

                def emit_score(c, h, kb, exp_sb):
                    q0 = max(0, kb * 128 - c * 512)
                    qcol = slice(q0, 512)
                    kc, kl = kb // 4, kb % 4
                    kcol = slice(kl * 128, (kl + 1) * 128)
                    qpe_l = (qpe_c[c][0:64, :] if h == 0 else qpe1_c[c])
                    ps = scps.tile([128, 512], F32, tag="sc")
                    nc.tensor.matmul(ps[:, q0:512], kn_c[kc][:, h, kcol],
                                     qn_c[c][:, h, qcol],
                                     start=True, stop=False)
                    nc.tensor.matmul(ps[:, q0:512], kpe_c[kc][:, kcol],
                                     qpe_l[:, qcol], start=False, stop=True)
                    if kb >= 4 * c:
                        # diagonal block: mask keys > query
                        nc.vector.tensor_add(ps[:, q0:q0 + 128],
                                             ps[:, q0:q0 + 128], diag_sb)
                    nc.scalar.activation(exp_sb[:, kb, q0:512], ps[:, q0:512],
                                         func=AF.Exp, scale=SCALING,
                                         bias=expb_sb)

                def emit_oproj_chain(attnT, tl, nch, osb):
                    ps = ops.tile([128, 512], F32, tag="op")
                    for h in range(HPC):
                        nc.tensor.matmul(
                            ps, attnT[:, h, tl * 128:(tl + 1) * 128],
                            wo[:, h, nch * 512:(nch + 1) * 512],
                            start=(h == 0), stop=(h == HPC - 1))
                    dst = osb[:, nch * 512:(nch + 1) * 512]
                    eng = "DADAPDADPA"[nch]
                    if eng == "D":
                        nc.vector.tensor_copy(dst, ps)
                    elif eng == "A":
                        nc.scalar.copy(dst, ps)
                    else:
                        nc.gpsimd.tensor_copy(dst, ps)

                def emit_store(c, tl, osb, split):
                    tt = 4 * c + tl
                    if split:
                        nc.sync.dma_start(
                            out=o_part[tt * 128:(tt + 1) * 128, 0:2560],
                            in_=osb[:, 0:2560])
                        nc.sync.dma_start(
                            out=o_part[tt * 128:(tt + 1) * 128, 2560:HID],
                            in_=osb[:, 2560:HID])
                    else:
                        nc.sync.dma_start(
                            out=o_part[tt * 128:(tt + 1) * 128, :], in_=osb)

                def emit_op(prev, c_prev, item):
                    tl, nch = item
                    emit_oproj_chain(prev[0], tl, nch, prev[1][tl])
                    if nch == HID // 512 - 1:
                        emit_store(c_prev, tl, prev[1][tl], False)

                def emit_pv_unit(c, h, tl, exp_sb, attnT):
                    tt = 4 * c + tl
                    pv = pvps.tile([128, NOPE + 1], F32, tag="pv")
                    for kb in range(tt + 1):
                        nc.tensor.matmul(
                            pv, exp_sb[:, kb, tl * 128:(tl + 1) * 128],
                            vt_c[kb // 4][:, kb % 4, h, :],
                            start=(kb == 0), stop=(kb == tt))
                    rcp = rcp_p.tile([128, 1], F32, tag="rcp")
                    nc.vector.reciprocal(rcp, pv[:, NOPE:NOPE + 1])
                    asb = asb_p.tile([128, 128], F16, tag="asb")
                    nc.scalar.activation(asb, pv[:, 0:NOPE],
                                         func=AF.Copy, scale=rcp)
                    tr = trps.tile([128, 128], F16, tag="tr")
                    nc.tensor.transpose(tr, asb, ident_sb)
                    nc.vector.tensor_copy(
                        attnT[:, h, tl * 128:(tl + 1) * 128], tr)

                prev = None  # (attnT, [osb x4]) of previous chunk
                for c in range(NCH):
                    exp_h = [expp.tile([128, TT, 512], F16, tag="exp",
                                        name=f"exp{_h}")
                             for _h in range(HPC)]
                    # merged stream: scores of chunk c + 1st half of o_proj
                    # of chunk c-1; 2nd half interleaves with the PV units.
                    sc_items = [(h, kb) for h in range(HPC)
                                for kb in range(4 * c + 4)]
                    all_ops = ([(tl, nch) for tl in range(4)
                                for nch in range(HID // 512)]
                               if prev is not None else [])
                    op1, op2 = all_ops[:20], all_ops[20:]
                    ns, no = len(sc_items), len(op1)
                    si = oi = 0
                    while si < ns or oi < no:
                        if si < ns and (no == 0 or si * no <= oi * ns):
                            h, kb = sc_items[si]; si += 1
                            emit_score(c, h, kb, exp_h[h])
                        else:
                            emit_op(prev, c - 1, op1[oi]); oi += 1
                    # PV with ones column: out = [attn | denom] per q-tile
                    attnT = attn_p.tile([128, HPC, 512], F16, tag="attnT")
                    pv_units = [(h, tl) for h in range(HPC)
                                for tl in range(4)]
                    nu, no = len(pv_units), len(op2)
                    ui = oi = 0
                    while ui < nu or oi < no:
                        if ui < nu and (no == 0 or ui * no <= oi * nu):
                            h, tl = pv_units[ui]; ui += 1
                            emit_pv_unit(c, h, tl, exp_h[h], attnT)
                        else:
                            emit_op(prev, c - 1, op2[oi]); oi += 1
                    prev = (attnT, [osb_p.tile([128, HID], F16, tag="osb",
                                                name=f"osb{_t}")
                                    for _t in range(4)])
                # trailing o_proj for the last chunk; stores split in halves
                # fired as soon as their columns are done
                for tl in range(4):
                    tt = 4 * (NCH - 1) + tl
                    osb = prev[1][tl]
                    cuts = ((2, 0, 1536), (4, 1536, 2560), (7, 2560, 4096),
                            (9, 4096, HID)) if tl == 3 else \
                           ((4, 0, 2560), (9, 2560, HID))
                    ci = 0
                    for nch in range(HID // 512):
                        emit_oproj_chain(prev[0], tl, nch, osb)
                        if ci < len(cuts) and nch == cuts[ci][0]:
                            _, a, b = cuts[ci]
                            ci += 1
                            nc.sync.dma_start(
                                out=o_part[tt * 128:(tt + 1) * 128, a:b],
                                in_=osb[:, a:b])
    nc.compile()
    return nc


_CACHE = {}


def _get(name):
    if name not in _CACHE:
        _CACHE[name] = _build_a() if name == "a" else _build_b()
    return _CACHE[name]


def _host_consts():
    ident = np.eye(128, dtype=np.float32)
    # swap matrix S: (Sx)[2i] = -x[2i+1], (Sx)[2i+1] = x[2i]; we pass S^T,
    # block-diag over the two 64-row head slots.
    st64 = np.zeros((64, 64), dtype=np.float32)
    for i in range(32):
        st64[2 * i, 2 * i + 1] = 1.0
        st64[2 * i + 1, 2 * i] = -1.0
    swap2t = np.zeros((128, 128), dtype=np.float32)
    swap2t[0:64, 0:64] = st64
    swap2t[64:128, 64:128] = st64
    r = np.arange(128)
    # diagT[r, c] = NEG where key r > query c
    diagT = np.where(r[:, None] > r[None, :], NEG, 0.0).astype(np.float32)
    return ident.astype(np.float16), swap2t.astype(np.float16), diagT


def _rope_tables(positions):
    # duplicated-pair (interleaved) layout, rows stacked twice for 2 heads
    inv_freq = 1.0 / (THETA ** (np.arange(0, ROPE, 2, dtype=np.float32)
                                / ROPE))
    freqs = positions.astype(np.float32)[:, None] * inv_freq[None, :]  # [T,32]
    cos = np.cos(freqs).astype(np.float32)
    sin = np.sin(freqs).astype(np.float32)
    cos_dup = np.repeat(cos, 2, axis=1).T.copy()   # [64, T]
    sin_dup = np.repeat(sin, 2, axis=1).T.copy()
    cos2 = np.vstack([cos_dup, cos_dup]).astype(np.float16)  # [128, T]
    sin2 = np.vstack([sin_dup, sin_dup]).astype(np.float16)
    return np.ascontiguousarray(cos2), np.ascontiguousarray(sin2)


def kernel(positions, hidden_states, w_fused, q_a_ln_w, kv_a_ln_w,
           w_qb, w_kvb, w_o):
    positions = np.asarray(positions)
    hidden_states = np.asarray(hidden_states, dtype=np.float32)
    w_fused = np.asarray(w_fused, dtype=np.float32)
    q_a_ln_w = np.ascontiguousarray(np.asarray(q_a_ln_w, dtype=np.float32))
    kv_a_ln_w = np.ascontiguousarray(np.asarray(kv_a_ln_w, dtype=np.float32))
    w_qb = np.asarray(w_qb, dtype=np.float32)
    w_kvb = np.asarray(w_kvb, dtype=np.float32)
    w_o = np.asarray(w_o, dtype=np.float32)

    ident, swap2t, diagT = _host_consts()
    w_fused16 = np.ascontiguousarray(w_fused.astype(np.float16))
    cos2, sin2 = _rope_tables(positions)

    # ---- launch A: sequence-parallel fused projection + norms
    nca = _get("a")
    in_a = []
    for c in range(NCORES):
        in_a.append({
            "hid_sT": np.ascontiguousarray(
                hidden_states[c * TS:(c + 1) * TS, :].T.astype(np.float16)),
            "w_fused": w_fused16,
            "q_ln": q_a_ln_w,
            "kv_ln": kv_a_ln_w,
        })
    res_a = bass_utils.run_bass_kernel_spmd(nca, in_a,
                                            core_ids=list(range(NCORES)))
    q_aT = np.concatenate([res_a.results[c]["q_aT_s"]
                           for c in range(NCORES)], axis=1)
    kv_aT = np.concatenate([res_a.results[c]["kv_aT_s"]
                            for c in range(NCORES)], axis=1)
    k_peT = np.concatenate([res_a.results[c]["k_peT_s"]
                            for c in range(NCORES)], axis=1)

    # ---- launch B: head-parallel attention
    ncb = _get("b")
    in_b = []
    for c in range(NCORES):
        g0, g1 = 2 * c, 2 * c + 1
        wq_s = np.ascontiguousarray(
            w_qb[:, g0 * (NOPE + ROPE):(g1 + 1) * (NOPE + ROPE)]
        ).astype(np.float16)
        wk = w_kvb
        wkv_s = np.ascontiguousarray(np.concatenate([
            wk[:, g0 * 256:g0 * 256 + 128],        # h0 nope
            wk[:, g1 * 256:g1 * 256 + 128],        # h1 nope
            wk[:, g0 * 256 + 128:(g0 + 1) * 256],  # h0 v
            wk[:, g1 * 256 + 128:(g1 + 1) * 256],  # h1 v
        ], axis=1)).astype(np.float16)
        wo_s = np.ascontiguousarray(
            w_o[g0 * VDIM:(g1 + 1) * VDIM, :]).astype(np.float16)
        in_b.append({
            "q_aT": q_aT, "kv_aT": kv_aT, "k_peT": k_peT,
            "w_qb_s": wq_s, "w_kvb_s": wkv_s, "w_o_s": wo_s,
            "cos2": cos2, "sin2": sin2,
            "swap2t": swap2t, "ident": ident, "diagT": diagT,
        })
    res_b = bass_utils.run_bass_kernel_spmd(ncb, in_b,
                                            core_ids=list(range(NCORES)))
    out = res_b.results[0]["o_part"].astype(np.float64)
    for c in range(1, NCORES):
        out += res_b.results[c]["o_part"]
    return out.astype(np.float32)
